# revision 32
# baseline (speedup 1.0000x reference)
"""GQA attention (int8-quantized QK^T, RoPE, causal softmax) on 8 TRN2 NeuronCores.

Sharding: tensor-parallel over heads. Core c owns Q heads 4c..4c+3 (Wq cols
512c..512c+512), KV head c (Wk/Wv cols 128c..128c+128), and Wo rows
512c..512c+512. x is replicated. Each core emits a partial [2048, 4096]
bf16 output (its heads' contribution through Wo); the host sums the 8
partials in float64. No on-device collectives.

Fully software-pipelined single pass (v2). Per-engine queue layout:
  PE      : proj(u0) proj(u1) T(u0) proj(u2) T(u1) proj(u3) T(u2) B(0)
            proj(u4) T(u3) proj(u5) T(u4) B(1)+C(0) ... B(3)+C(2) C(3)
  Vector  : rope/quant per s-tile; dequant STT pairs; softmax-den recip;
            O^T normalize; half of C PSUM evac
  Scalar  : weight fp32->bf16 casts; QKV PSUM evac; qT/kT/qsr evacs;
            exp on [128,1024] pairs; half of C evac
  GpSimd  : SWDGE x casts (fp32->bf16 DRAM->DRAM, chunk-major) + out
            writes; q-scale broadcasts; causal masks; den broadcasts
  Sync    : hardware DMA-transposes of x (contiguous chunk-major source)
  ScalarQ : staged fp32 weight loads (HWDGE)

Numerics identical to the baseline kernel (bf16 matmuls; QK^T exact in
int-valued bf16 with fp32 PSUM accumulation; MAGIC-constant round-to-
nearest-even; bf16 q-scales).
"""

import numpy as np

import concourse.bass as bass
import concourse.mybir as mybir
import concourse.tile as tile
from concourse import bacc
from concourse.bass_utils import run_bass_kernel_spmd
from concourse.masks import make_identity

FP = mybir.dt.float32
BF = mybir.dt.bfloat16
AL = mybir.AluOpType
AF = mybir.ActivationFunctionType

B, S, D, NH, NKV, HD = 1, 2048, 4096, 32, 8, 128
NCORES = 8
HPC = NH // NCORES          # 4 Q heads per core
FQ = HPC * HD               # 512
SCALE = HD ** -0.5
MAGIC = 3 * 2.0 ** 22       # fp32 round-to-nearest-even magic constant

ST = S // 128               # 16 s-tiles of 128 rows
DC = D // 128               # 32 d-chunks
NJ = S // 512               # 4 q-blocks of 512
NU = 8                      # x cast/transpose units of 256 rows (2 s-tiles)
NHK = HPC + 1               # 4 q heads + 1 k head, fused rope/quant


def build_graph():
    nc = bacc.Bacc(None)
    x_e = nc.declare_dram_parameter("x", [S, D], FP, isOutput=False)
    wq_e = nc.declare_dram_parameter("wq", [D, FQ], FP, isOutput=False)
    wk_e = nc.declare_dram_parameter("wk", [D, HD], FP, isOutput=False)
    wv_e = nc.declare_dram_parameter("wv", [D, HD], FP, isOutput=False)
    wo_e = nc.declare_dram_parameter("wo", [FQ, D], FP, isOutput=False)
    cos_e = nc.declare_dram_parameter("cos", [S, HD], FP, isOutput=False)
    sin_e = nc.declare_dram_parameter("sin", [S, HD], FP, isOutput=False)
    out_e = nc.declare_dram_parameter("out", [S, D], BF, isOutput=True)
    import os
    DEBUG = os.environ.get("KDEBUG", "0") == "1"
    if DEBUG:
        dbg_wq = nc.declare_dram_parameter("dbg_wq", [128, DC, FQ], BF, isOutput=True)
        dbg_wkv = nc.declare_dram_parameter("dbg_wkv", [128, DC, 2 * HD], BF, isOutput=True)
        dbg_wo = nc.declare_dram_parameter("dbg_wo", [128, HPC, D], BF, isOutput=True)
        dbg_kT = nc.declare_dram_parameter("dbg_kT", [128, S], BF, isOutput=True)
        dbg_vn = nc.declare_dram_parameter("dbg_vn", [128, ST, HD], BF, isOutput=True)
        dbg_ksr = nc.declare_dram_parameter("dbg_ksr", [128, ST], FP, isOutput=True)
        dbg_xt = nc.declare_dram_parameter("dbg_xt", [128, DC, 256], BF, isOutput=True)
        dbg_xdr = nc.declare_dram_parameter("dbg_xdr", [DC, 512, 128], BF, isOutput=True)

    from contextlib import ExitStack
    with tile.TileContext(nc, pool_alloc_mode="queue") as tc:
        with ExitStack() as stack:
            pool = lambda *a, **kw: stack.enter_context(tc.tile_pool(*a, **kw))
            pp = pool(name="persist", bufs=1)
            wp = pool(name="wts", bufs=1)
            wsp = pool(name="wstage", bufs=2)
            xtp = pool(name="xt", bufs=2)
            qtp = pool(name="qt", bufs=2)
            otp = pool(name="ot", bufs=2)
            csp = pool(name="cs", bufs=2)
            qkvp = pool(name="qkv", bufs=2)
            rpp = pool(name="rope", bufs=1)
            qip = pool(name="qi", bufs=4)
            qsp = pool(name="qsr", bufs=2)
            atp = pool(name="att", bufs=2)
            dnp = pool(name="den", bufs=1)
            ptp = pool(name="ptp", bufs=3)
            cop = pool(name="cout", bufs=2)
            drp = pool(name="dram", bufs=1, space="DRAM")
            ps_qkv = pool(name="psqkv", bufs=2, space="PSUM")
            ps_sc = pool(name="pssc", bufs=1, space="PSUM")
            ps_ot = pool(name="psot", bufs=1, space="PSUM")
            ps_m = pool(name="psmisc", bufs=1, space="PSUM")
            ps_c = pool(name="psc", bufs=2, space="PSUM")
            # ---- constants FIRST (gpsimd queue must not sit behind DMAs)
            ident = pp.tile([128, 128], BF)
            make_identity(nc, ident[:])
            ones1 = pp.tile([128, 1], BF)
            nc.gpsimd.memset(ones1[:], 1.0)

            # persistent state
            kT = pp.tile([128, S], BF)          # quantized K^T
            vn = pp.tile([128, ST, HD], BF)     # V natural per t-tile
            ksr = pp.tile([128, ST], FP)        # k dequant scale (SCALE folded)

            # persistent bf16 weights
            wqr = wp.tile([128, DC, FQ], BF)
            wkv = wp.tile([128, DC, 2 * HD], BF)
            wo_r = wp.tile([128, HPC, D], BF)

            # chunk-major bf16 scratch for x. Two DRAM tiles per unit
            # (d-chunks 0-15 / 16-31): each strided cast is consumed by
            # exactly ONE HWDGE ring's transposes — a single waiting ring
            # per multi-part SWDGE cast (two rings waiting on one strided
            # cast starts transposing before the cast lands).
            xdrA, xdrB = [], []
            for u in range(NU):
                xdrA.append(drp.tile([DC // 2, 256, 128], BF, tag=f"xdra{u}",
                                     name=f"xdra{u}"))
                xdrB.append(drp.tile([DC // 2, 256, 128], BF, tag=f"xdrb{u}",
                                     name=f"xdrb{u}"))

            # ---- HWDGE queue round-robin (sync/scalar) for transposes,
            # weight fp32 loads and cos/sin; weight casts on VectorE.
            def hw_eng():
                return nc.scalar

            wq_v = wq_e[:].rearrange("(c p) f -> p c f", p=128)   # [128, DC, FQ]
            wkv_vk = wk_e[:].rearrange("(c p) h -> p c h", p=128)
            wkv_vv = wv_e[:].rearrange("(c p) h -> p c h", p=128)
            wo_v = wo_e[:].rearrange("(f p) d -> p f d", p=128)   # [128, HPC, D]

            # Weight-load closures drained between transpose ops so neither
            # HWDGE queue is monopolized.
            pending_w = []

            def make_wq(i):
                def go():
                    st_t = wsp.tile([128, 2, FQ], FP, tag="wst", name="wst")
                    hw_eng().dma_start(st_t[:], wq_v[:, 2 * i:2 * i + 2, :])
                    nc.vector.tensor_copy(wqr[:, 2 * i:2 * i + 2, :], st_t[:])
                return go

            def make_wkv(i):
                def go():
                    st_t = wsp.tile([128, 4, 2 * HD], FP, tag="wst", name="wst")
                    hw_eng().dma_start(st_t[:, :, 0:HD], wkv_vk[:, 4 * i:4 * i + 4, :])
                    hw_eng().dma_start(st_t[:, :, HD:2 * HD],
                                       wkv_vv[:, 4 * i:4 * i + 4, :])
                    nc.vector.tensor_copy(wkv[:, 4 * i:4 * i + 4, :], st_t[:])
                return go

            def make_wo(i):
                f, hh = divmod(i, 4)

                def go():
                    st_t = wsp.tile([128, 1, D // 4], FP, tag="wst", name="wst")
                    hw_eng().dma_start(
                        st_t[:], wo_v[:, f:f + 1, hh * (D // 4):(hh + 1) * (D // 4)])
                    nc.vector.tensor_copy(
                        wo_r[:, f:f + 1, hh * (D // 4):(hh + 1) * (D // 4)], st_t[:])
                return go

            for i in range(DC // 2):
                pending_w.append(make_wq(i))
            for i in range(DC // 4):
                pending_w.append(make_wkv(i))
            for i in range(16):
                pending_w.append(make_wo(i))

            def drain_w(n):
                for _ in range(min(n, len(pending_w))):
                    pending_w.pop(0)()

            # ---- x casts: SWDGE DRAM->DRAM fp32->bf16, chunk-major dst.
            # First three units up front; the rest spread between A-units.
            def emit_xcast(u):
                nc.gpsimd.dma_start(
                    xdrA[u][:],
                    x_e[u * 256:(u + 1) * 256, 0:D // 2].rearrange(
                        "r (c k) -> c r k", k=128))
                nc.gpsimd.dma_start(
                    xdrB[u][:],
                    x_e[u * 256:(u + 1) * 256, D // 2:D].rearrange(
                        "r (c k) -> c r k", k=128))

            for u in range(3):
                emit_xcast(u)

            # cos/sin chunked per unit: [128, 2, HD] fp32 (rows 128*2)
            def emit_cs(u):
                cs_c = csp.tile([128, 2, HD], FP, tag="cosc")
                cs_s = csp.tile([128, 2, HD], FP, tag="sinc")
                r0 = u * 256
                hw_eng().dma_start(
                    cs_c[:], cos_e[r0:r0 + 256, :].rearrange("(t p) d -> p t d", p=128))
                hw_eng().dma_start(
                    cs_s[:], sin_e[r0:r0 + 256, :].rearrange("(t p) d -> p t d", p=128))
                # [-sin | +sin] halves
                nc.vector.tensor_scalar_mul(cs_s[:, :, 0:64], cs_s[:, :, 0:64], -1.0)
                return cs_c, cs_s

            # xT tiles per unit [128, DC, 256]; alternate the two HWDGE
            # queues per chunk and drain a weight load every few chunks.
            def emit_transposes(u, xT_u):
                # weight loads first: they have no deps, so they fill the
                # HWDGE queues while the unit's x-cast is still in flight
                drain_w(24 if u == 0 else 16)
                for c in range(DC // 2):
                    nc.sync.dma_start(xT_u[:, c, :], xdrA[u][c],
                                      transpose=True)
                    nc.sync.dma_start(xT_u[:, DC // 2 + c, :], xdrB[u][c],
                                      transpose=True)

            # ---- A-unit: projections + rope + quant for 2 s-tiles
            def emit_A_unit(u, xT_u, cs_c, cs_s):
                for i in range(2):
                    st_i = 2 * u + i
                    q_ps = ps_qkv.tile([128, FQ], FP, tag="qkvps")
                    kv_ps = ps_qkv.tile([128, 2 * HD], FP, tag="qkvps")
                    for d in range(DC):
                        nc.tensor.matmul(q_ps[:], xT_u[:, d, i * 128:(i + 1) * 128],
                                         wqr[:, d, :],
                                         start=(d == 0), stop=(d == DC - 1))
                    for d in range(DC):
                        nc.tensor.matmul(kv_ps[:], xT_u[:, d, i * 128:(i + 1) * 128],
                                         wkv[:, d, :],
                                         start=(d == 0), stop=(d == DC - 1))
                    # evacuate PSUM promptly (frees banks, SBUF-src rope)
                    qk_sb = qkvp.tile([128, FQ + 2 * HD], FP, tag="qkvsb")
                    nc.scalar.copy(qk_sb[:, 0:FQ], q_ps[:])
                    nc.scalar.copy(qk_sb[:, FQ:FQ + 2 * HD], kv_ps[:])
                    # V natural: straight cast
                    nc.vector.tensor_copy(vn[:, st_i, :], qk_sb[:, FQ + HD:FQ + 2 * HD])

                    # RoPE + absmax quant, 5 heads fused (4 q + 1 k)
                    rr = rpp.tile([128, NHK, HD], FP, tag="rr")
                    t2 = rpp.tile([128, NHK, HD], FP, tag="t2")
                    am = rpp.tile([128, NHK], FP, tag="am")
                    sc = rpp.tile([128, NHK], FP, tag="sc")
                    co = cs_c[:, i, :]
                    si = cs_s[:, i, :]
                    for h in range(NHK):
                        src = qk_sb[:, h * HD:(h + 1) * HD]
                        nc.vector.tensor_mul(rr[:, h, :], src, co)
                        nc.vector.tensor_mul(t2[:, h, 0:64],
                                             qk_sb[:, h * HD + 64:(h + 1) * HD],
                                             si[:, 0:64])
                        nc.vector.tensor_mul(t2[:, h, 64:HD],
                                             qk_sb[:, h * HD:h * HD + 64],
                                             si[:, 64:HD])
                    nc.vector.tensor_add(rr[:], rr[:], t2[:])
                    nc.vector.tensor_reduce(am[:], rr[:], axis=mybir.AxisListType.X,
                                            op=AL.max, apply_absolute_value=True)
                    nc.vector.tensor_scalar_max(am[:], am[:], 1e-5)
                    nc.vector.reciprocal_approx_fast(sc[:], am[:])
                    for h in range(NHK):
                        nc.vector.tensor_scalar(rr[:, h, :], rr[:, h, :],
                                                sc[:, h:h + 1], None, op0=AL.mult)
                    qi = qip.tile([128, NHK * HD], BF, tag="qi")
                    nc.vector.tensor_scalar(rr[:], rr[:], 127.0, MAGIC,
                                            op0=AL.mult, op1=AL.add)
                    nc.vector.tensor_scalar(qi[:], rr[:], MAGIC, None,
                                            op0=AL.subtract)
                    # dequant scale rows
                    am_bf = qip.tile([128, HPC], BF, tag="ambf")
                    nc.vector.tensor_scalar_mul(am_bf[:], am[:, 0:HPC], 1.0 / 127.0)
                    nc.vector.tensor_scalar_mul(ksr[:, st_i:st_i + 1],
                                                am[:, HPC:HPC + 1], SCALE / 127.0)
                    yield st_i, qi, am_bf

            # PE transposes for one s-tile (deferred one unit for PE density)
            def emit_T_stile(st_i, qi, am_bf, qT_blk, qsr_blk):
                for h in range(HPC):
                    tp = ps_m.tile([128, 128], BF, tag="misc")
                    nc.tensor.transpose(tp[:], qi[:, h * HD:(h + 1) * HD], ident[:])
                    nc.scalar.copy(qT_blk[:, h, (st_i % 4) * 128:(st_i % 4 + 1) * 128],
                                   tp[:])
                tp = ps_m.tile([128, 128], BF, tag="misc")
                nc.tensor.transpose(tp[:], qi[:, HPC * HD:NHK * HD], ident[:])
                nc.scalar.copy(kT[:, st_i * 128:(st_i + 1) * 128], tp[:])
                for h in range(HPC):
                    tq = ps_m.tile([1, 128], BF, tag="misc")
                    nc.tensor.transpose(tq[:], am_bf[:, h:h + 1], ident[:])
                    nc.scalar.copy(
                        qsr_blk[0:1, h, (st_i % 4) * 128:(st_i % 4 + 1) * 128],
                        tq[:])

            # ---- B block: attention for q-block J (t-tiles 0..4J+3)
            # C(J-1) s-tiles are interleaved by the caller between heads.
            def emit_B_head(J, h, qT_blk, qsr_blk, OT_blk):
                NL = 4 * J + 4
                NP = NL // 2
                dqb = atp.tile([128, 512], BF, tag="dqb")
                nc.gpsimd.partition_broadcast(dqb[:], qsr_blk[0:1, h, :])
                oT_ps = ps_ot.tile([128, 512], FP, tag="o")
                den_ps = ps_m.tile([1, 512], FP, tag="misc")
                sc_ps = ps_sc.tile([128, 2, 512], FP, tag="sc")

                pts = [None] * NP

                def emit_denot(p):
                    pt = pts[p]
                    for half in range(2):
                        ti = 2 * p + half
                        nc.tensor.matmul(den_ps[:], ones1[:], pt[:, half, :],
                                         start=(ti == 0), stop=(ti == NL - 1))
                        nc.tensor.matmul(oT_ps[:], vn[:, ti, :], pt[:, half, :],
                                         start=(ti == 0), stop=(ti == NL - 1))

                for p in range(NP):
                    for half in range(2):
                        ti = 2 * p + half
                        nc.tensor.matmul(sc_ps[:, half, :],
                                         kT[:, ti * 128:(ti + 1) * 128],
                                         qT_blk[:, h, :])
                    if p >= 2:
                        emit_denot(p - 2)
                    ptf = atp.tile([128, 2, 512], FP, tag="ptf")
                    for half in range(2):
                        ti = 2 * p + half
                        nc.vector.scalar_tensor_tensor(
                            out=ptf[:, half, :], in0=sc_ps[:, half, :],
                            scalar=ksr[:, ti:ti + 1],
                            in1=dqb[:], op0=AL.mult, op1=AL.mult)
                    pt = ptp.tile([128, 2, 512], BF, tag="pt")
                    nc.scalar.activation(pt[:], ptf[:], AF.Exp)
                    for half in range(2):
                        ti = 2 * p + half
                        if ti >= 4 * J:
                            nc.gpsimd.affine_select(
                                out=pt[:, half, :], in_=pt[:, half, :],
                                compare_op=AL.is_ge, fill=0.0,
                                base=J * 512 - ti * 128, channel_multiplier=-1,
                                pattern=[[1, 512]])
                    pts[p] = pt
                for p in range(max(0, NP - 2), NP):
                    emit_denot(p)

                denr = dnp.tile([1, 512], FP, tag="denr")
                nc.vector.reciprocal_approx_fast(denr[:], den_ps[:])
                dnb = dnp.tile([128, 512], FP, tag="dnb")
                nc.gpsimd.partition_broadcast(dnb[:], denr[:])
                nc.vector.tensor_mul(OT_blk[:, h, :], oT_ps[:], dnb[:])

            # ---- C: output projection, one quarter-s-tile per closure so
            # the PE work interleaves finely with B's attention pairs.
            pending_C = []

            def make_C_qtr(st_i, qtr, OT_blk):
                def go():
                    ot_sb = cop.tile([128, D // 4], BF, tag="otsb", name="otsb")
                    for dbl in range(2):
                        db = qtr * 2 + dbl
                        wo_ps = ps_c.tile([128, 512], FP, tag="wo", name="wo")
                        for f in range(HPC):
                            nc.tensor.matmul(
                                wo_ps[:],
                                OT_blk[:, f, (st_i % 4) * 128:(st_i % 4 + 1) * 128],
                                wo_r[:, f, db * 512:(db + 1) * 512],
                                start=(f == 0), stop=(f == HPC - 1))
                        if db % 2 == 0:
                            nc.scalar.copy(ot_sb[:, dbl * 512:(dbl + 1) * 512],
                                           wo_ps[:])
                        else:
                            nc.vector.tensor_copy(ot_sb[:, dbl * 512:(dbl + 1) * 512],
                                                  wo_ps[:])
                    nc.gpsimd.dma_start(
                        out_e[st_i * 128:(st_i + 1) * 128,
                              qtr * (D // 4):(qtr + 1) * (D // 4)],
                        ot_sb[:])
                return go

            def queue_C_stile(st_i, OT_blk):
                for qtr in range(4):
                    pending_C.append(make_C_qtr(st_i, qtr, OT_blk))

            def drain_C(n):
                for _ in range(min(n, len(pending_C))):
                    pending_C.pop(0)()

            # =============== emission schedule ===============
            pending_T = []          # deferred transposes (one unit behind)
            qT_blks = {}
            qsr_blks = {}
            OT_blks = {}

            def run_A_unit(u, qT_blk, qsr_blk):
                cs_c, cs_s = emit_cs(u)
                xT_u = xtp.tile([128, DC, 256], BF, tag="xt")
                emit_transposes(u, xT_u)
                if DEBUG and u == 0:
                    nc.gpsimd.dma_start(dbg_xt[:], xT_u[:])
                new_T = []
                for st_i, qi, am_bf in emit_A_unit(u, xT_u, cs_c, cs_s):
                    new_T.append((st_i, qi, am_bf, qT_blk, qsr_blk))
                # flush previous unit's PE transposes after this unit's MMs
                for args in pending_T[:]:
                    emit_T_stile(*args)
                pending_T.clear()
                pending_T.extend(new_T)

            # wo staging: emitted after unit 1 (scalar queue, spread)
            def new_blk(J):
                qT_blks[J] = qtp.tile([128, HPC, 512], BF, tag="qtb", name="qtb")
                qsr_blks[J] = qsp.tile([1, HPC, 512], BF, tag="qsrb", name="qsrb")

            # prologue: units 0,1
            new_blk(0)
            run_A_unit(0, qT_blks[0], qsr_blks[0])
            run_A_unit(1, qT_blks[0], qsr_blks[0])
            drain_w(len(pending_w))

            for J in range(NJ):
                OT_blks[J] = otp.tile([128, HPC, 512], BF, tag="otb", name="otb")
                if J >= 1:
                    for st in range(4):
                        queue_C_stile(4 * (J - 1) + st, OT_blks[J - 1])
                if J < NJ - 1:
                    new_blk(J + 1)
                    emit_xcast(2 * J + 3)
                    # A(2J+2) flushes T(unit 2J+1), completing qT/kT for B(J)
                    run_A_unit(2 * J + 2, qT_blks[J + 1], qsr_blks[J + 1])
                    emit_B_head(J, 0, qT_blks[J], qsr_blks[J], OT_blks[J])
                    drain_C(2)
                    emit_B_head(J, 1, qT_blks[J], qsr_blks[J], OT_blks[J])
                    drain_C(2)
                    if 2 * J + 4 < NU:
                        emit_xcast(2 * J + 4)
                    run_A_unit(2 * J + 3, qT_blks[J + 1], qsr_blks[J + 1])
                    emit_B_head(J, 2, qT_blks[J], qsr_blks[J], OT_blks[J])
                    drain_C(2)
                    emit_B_head(J, 3, qT_blks[J], qsr_blks[J], OT_blks[J])
                    drain_C(2)
                else:
                    for args in pending_T[:]:
                        emit_T_stile(*args)
                    pending_T.clear()
                    for h in range(HPC):
                        emit_B_head(J, h, qT_blks[J], qsr_blks[J], OT_blks[J])
                        drain_C(4)
            drain_C(len(pending_C))
            for st in range(4):
                queue_C_stile(4 * (NJ - 1) + st, OT_blks[NJ - 1])
            drain_C(len(pending_C))
            if DEBUG:
                nc.gpsimd.dma_start(dbg_xdr[0:DC // 2, 0:256, :], xdrA[0][:])
                nc.gpsimd.dma_start(dbg_xdr[DC // 2:DC, 0:256, :], xdrB[0][:])
                nc.gpsimd.dma_start(dbg_wq[:], wqr[:])
                nc.gpsimd.dma_start(dbg_wkv[:], wkv[:])
                nc.gpsimd.dma_start(dbg_wo[:], wo_r[:])
                nc.gpsimd.dma_start(dbg_kT[:], kT[:])
                nc.gpsimd.dma_start(dbg_vn[:], vn[:])
                nc.gpsimd.dma_start(dbg_ksr[:], ksr[:])

    nc.compile()
    return nc


_CACHE = {}


def kernel(x, Wq, Wk, Wv, Wo, cos, sin):
    x2 = np.ascontiguousarray(np.asarray(x, np.float32).reshape(S, D))
    in_maps = []
    for c in range(NCORES):
        in_maps.append({
            "x": x2,
            "wq": np.ascontiguousarray(Wq[:, c * FQ:(c + 1) * FQ], np.float32),
            "wk": np.ascontiguousarray(Wk[:, c * HD:(c + 1) * HD], np.float32),
            "wv": np.ascontiguousarray(Wv[:, c * HD:(c + 1) * HD], np.float32),
            "wo": np.ascontiguousarray(Wo[c * FQ:(c + 1) * FQ, :], np.float32),
            "cos": np.ascontiguousarray(cos, np.float32),
            "sin": np.ascontiguousarray(sin, np.float32),
        })
    if "nc" not in _CACHE:
        _CACHE["nc"] = build_graph()
    try:
        res = run_bass_kernel_spmd(_CACHE["nc"], in_maps, core_ids=list(range(NCORES)))
    except Exception:
        # transient NRT/device hiccups (e.g. EXEC_UNIT_UNRECOVERABLE) usually
        # clear on a fresh attempt
        import time
        time.sleep(20)
        res = run_bass_kernel_spmd(_CACHE["nc"], in_maps, core_ids=list(range(NCORES)))
    out = np.zeros((S, D), np.float64)
    for r in res.results:
        out += np.asarray(r["out"], np.float64)
    return out.astype(np.float32).reshape(B, S, D)


# revision 33
# speedup vs baseline: 1.6182x; 1.6182x over previous
"""GQA attention (int8-quantized QK^T, RoPE, causal softmax) on 8 TRN2 NeuronCores.

Sharding: tensor-parallel over heads. Core c owns Q heads 4c..4c+3 (Wq cols
512c..512c+512), KV head c (Wk/Wv cols 128c..128c+128), and Wo rows
512c..512c+512. x is replicated. Each core emits a partial [2048, 4096]
bf16 output (its heads' contribution through Wo); the host sums the 8
partials in float64. No on-device collectives.

Per-core dataflow (matmuls in bf16; QK^T is exact: int-quantized values are
integers <= 127, exactly representable in bf16, accumulated in fp32 PSUM):
  A) x -> bf16 DRAM scratch (column-chunked casts) -> x^T via hardware
     DMA-transpose on the Sync queue; Q/K/V projections in natural [s, f]
     layout; RoPE + absmax-quantize on VectorE; PE-transpose q/k to [hd, s].
  B) scores^T [t, q] = kT-slice.T @ qT-block; dequant via
     scalar_tensor_tensor (k-scale per-partition, q-scale broadcast from
     gpsimd partition_broadcast); exp on ScalarE; causal zeroing of
     diagonal-band tiles on gpsimd post-exp; den = ones.T @ P^T;
     O^T += V-chunk.T @ P^T; heads processed in pairs so TensorE always has
     the sibling head's matmuls while one head's dequant/exp round-trips.
  C) out[s, :] += OT-slice.T @ Wo-chunk accumulated over f, DMA out (bf16)
     on the Scalar queue.
"""

import numpy as np

import concourse.bass as bass
import concourse.mybir as mybir
import concourse.tile as tile
from concourse import bacc
from concourse.bass_utils import run_bass_kernel_spmd
from concourse.masks import make_identity

FP = mybir.dt.float32
BF = mybir.dt.bfloat16
AL = mybir.AluOpType
AF = mybir.ActivationFunctionType

B, S, D, NH, NKV, HD = 1, 2048, 4096, 32, 8, 128
NCORES = 8
HPC = NH // NCORES          # 4 Q heads per core
FQ = HPC * HD               # 512
SCALE = HD ** -0.5
MAGIC = 3 * 2.0 ** 22       # fp32 round-to-nearest-even magic constant

ST = S // 128               # 16 s-tiles of 128 rows
DC = D // 128               # 32 d-chunks
NJ = S // 512               # 4 q-blocks of 512
SBLK = 4                    # s-tiles per x-transpose block (512 rows)
NB = ST // SBLK


def build_graph():
    nc = bacc.Bacc(None)
    x_e = nc.declare_dram_parameter("x", [S, D], FP, isOutput=False)
    wq_e = nc.declare_dram_parameter("wq", [D, FQ], FP, isOutput=False)
    wk_e = nc.declare_dram_parameter("wk", [D, HD], FP, isOutput=False)
    wv_e = nc.declare_dram_parameter("wv", [D, HD], FP, isOutput=False)
    wo_e = nc.declare_dram_parameter("wo", [FQ, D], FP, isOutput=False)
    cos_e = nc.declare_dram_parameter("cos", [S, HD], FP, isOutput=False)
    sin_e = nc.declare_dram_parameter("sin", [S, HD], FP, isOutput=False)
    out_e = nc.declare_dram_parameter("out", [S, D], BF, isOutput=True)

    with tile.TileContext(nc, pool_alloc_mode="queue") as tc:
        with (
            tc.tile_pool(name="persist", bufs=1) as pp,
        ):
            ident = pp.tile([128, 128], BF)
            make_identity(nc, ident[:])
            ones1 = pp.tile([128, 1], BF)       # den stationary (M=1)
            nc.gpsimd.memset(ones1[:], 1.0)

            qT = pp.tile([128, HPC, S], BF)     # quantized Q^T per head
            kT = pp.tile([128, S], BF)          # quantized K^T
            vn = pp.tile([128, ST, HD], BF)     # V natural, per t-chunk
            qsrT = pp.tile([1, HPC, S], BF)     # q dequant scale rows (partition 0)
            ksr = pp.tile([128, ST], FP)        # k dequant scale (SCALE folded)
            OT = pp.tile([128, HPC, S], BF)     # normalized O^T per head

            # ---------------- Phase A: x^T, projections, RoPE, quantize
            with (
                tc.tile_pool(name="ropec", bufs=1) as rp,
                tc.tile_pool(name="xtp", bufs=2) as xtp,
                tc.tile_pool(name="dram", bufs=1, space="DRAM") as drp,
                tc.tile_pool(name="wq", bufs=1) as wqp,
                tc.tile_pool(name="ab", bufs=2) as ab,
                tc.tile_pool(name="psA", bufs=2, space="PSUM") as psA,
                tc.tile_pool(name="psA1", bufs=2, space="PSUM") as psA1,
            ):
                # SWDGE queue order == emission order. x casts are
                # column-chunked so each block's transposes can begin after
                # one quarter of its cast lands.
                xdrs = []
                for blk in range(NB):
                    xdr = drp.tile([SBLK * 128, D], BF, tag=f"xdr{blk}")
                    xdrs.append(xdr)
                nc.gpsimd.dma_start(xdrs[0][:], x_e[0:SBLK * 128, :])
                wqr = wqp.tile([128, DC, FQ], BF)
                for wc in range(4):
                    nc.gpsimd.dma_start(
                        wqr[:, wc * 8:(wc + 1) * 8, :],
                        wq_e[:].rearrange("(c p) f -> p c f", p=128)[:, wc * 8:(wc + 1) * 8, :])
                wkv = rp.tile([128, DC, 2 * HD], BF)
                nc.gpsimd.dma_start(wkv[:, :, 0:HD], wk_e[:].rearrange("(c p) h -> p c h", p=128))
                nc.gpsimd.dma_start(wkv[:, :, HD:2 * HD], wv_e[:].rearrange("(c p) h -> p c h", p=128))
                for blk in range(1, NB):
                    r0 = blk * SBLK * 128
                    nc.gpsimd.dma_start(xdrs[blk][:], x_e[r0:r0 + SBLK * 128, :])

                cosr = rp.tile([128, ST, HD], FP)
                sinm = rp.tile([128, ST, HD], FP)   # [-sin | +sin] halves
                nc.sync.dma_start(cosr[:], cos_e[:].rearrange("(t p) d -> p t d", p=128))
                nc.sync.dma_start(sinm[:], sin_e[:].rearrange("(t p) d -> p t d", p=128))
                nc.vector.tensor_scalar_mul(sinm[:, :, 0:64], sinm[:, :, 0:64], -1.0)

                for blk in range(NB):
                    xTs = []
                    for d in range(DC):
                        xTd = xtp.tile([128, SBLK * 128], BF, tag=f"xT{d}")
                        xTs.append(xTd)
                    for d in range(DC):
                        nc.sync.dma_start(
                            xTs[d][:],
                            xdrs[blk][:, d * 128:(d + 1) * 128],
                            transpose=True)

                    for i in range(SBLK):
                        st_i = blk * SBLK + i
                        q_ps = psA.tile([128, FQ], FP, tag="qps")
                        kv_ps = psA.tile([128, 2 * HD], FP, tag="kvps")
                        for d in range(DC):
                            nc.tensor.matmul(q_ps[:], xTs[d][:, i * 128:(i + 1) * 128],
                                             wqr[:, d, :],
                                             start=(d == 0), stop=(d == DC - 1))
                        for d in range(DC):
                            nc.tensor.matmul(kv_ps[:], xTs[d][:, i * 128:(i + 1) * 128],
                                             wkv[:, d, :],
                                             start=(d == 0), stop=(d == DC - 1))

                        # V natural: straight cast
                        nc.scalar.copy(vn[:, st_i, :], kv_ps[:, HD:2 * HD])

                        # RoPE + quantize q (4 heads) and k (1 head)
                        qi = ab.tile([128, FQ], BF, tag="qi")
                        ki = ab.tile([128, HD], BF, tag="ki")
                        for (src, nh, i8out) in ((q_ps, HPC, qi), (kv_ps, 1, ki)):
                            rr = ab.tile([128, nh, HD], FP, tag=f"rr{nh}")
                            t2 = ab.tile([128, nh, HD], FP, tag=f"t2{nh}")
                            am = ab.tile([128, nh], FP, tag=f"am{nh}")
                            sc = ab.tile([128, nh], FP, tag=f"sc{nh}")
                            for h in range(nh):
                                co = cosr[:, st_i, :]
                                si = sinm[:, st_i, :]
                                nc.vector.tensor_mul(rr[:, h, :], src[:, h * HD:(h + 1) * HD], co)
                                nc.vector.tensor_mul(t2[:, h, 0:64], src[:, h * HD + 64:(h + 1) * HD], si[:, 0:64])
                                nc.vector.tensor_mul(t2[:, h, 64:HD], src[:, h * HD:h * HD + 64], si[:, 64:HD])
                            nc.vector.tensor_add(rr[:], rr[:], t2[:])
                            nc.vector.tensor_reduce(am[:], rr[:], axis=mybir.AxisListType.X,
                                                    op=AL.max, apply_absolute_value=True)
                            nc.vector.tensor_scalar_max(am[:], am[:], 1e-5)
                            nc.vector.reciprocal_approx_fast(sc[:], am[:])   # ~1/amax
                            for h in range(nh):
                                nc.vector.tensor_scalar(rr[:, h, :], rr[:, h, :],
                                                        sc[:, h:h + 1], None, op0=AL.mult)
                            nc.vector.tensor_scalar(rr[:], rr[:], 127.0, MAGIC, op0=AL.mult, op1=AL.add)
                            nc.vector.tensor_scalar(i8out[:], rr[:], MAGIC, None, op0=AL.subtract)
                            if nh == 1:
                                nc.vector.tensor_scalar_mul(ksr[:, st_i:st_i + 1], am[:], SCALE / 127.0)
                            else:
                                am_bf = ab.tile([128, HPC], BF, tag="ambf")
                                nc.vector.tensor_scalar_mul(am_bf[:], am[:], 1.0 / 127.0)
                                for h in range(HPC):
                                    qsr_ps = psA1.tile([1, 128], BF, tag="qsrtp")
                                    nc.tensor.transpose(qsr_ps[:], am_bf[:, h:h + 1], ident[:])
                                    nc.scalar.copy(qsrT[0:1, h, st_i * 128:(st_i + 1) * 128],
                                                   qsr_ps[:])

                        # transpose quantized q/k into [hd, s] layout via PE
                        for h in range(HPC):
                            tp = psA.tile([128, 128], BF, tag="tp")
                            nc.tensor.transpose(tp[:], qi[:, h * HD:(h + 1) * HD], ident[:])
                            nc.scalar.copy(qT[:, h, st_i * 128:(st_i + 1) * 128], tp[:])
                        tp = psA.tile([128, 128], BF, tag="tp")
                        nc.tensor.transpose(tp[:], ki[:], ident[:])
                        nc.scalar.copy(kT[:, st_i * 128:(st_i + 1) * 128], tp[:])

            # ---------------- Phase B: attention (Wo prefetched meanwhile)
            wop_cm = tc.tile_pool(name="wo", bufs=1)
            wop = wop_cm.__enter__()
            wo_r = wop.tile([128, HPC, D], BF)
            nc.gpsimd.dma_start(wo_r[:], wo_e[:].rearrange("(f p) d -> p f d", p=128))
            with (
                tc.tile_pool(name="att", bufs=3) as at,
                tc.tile_pool(name="attf", bufs=4) as atf,
                tc.tile_pool(name="psSC", bufs=3, space="PSUM") as psSC,
                tc.tile_pool(name="psO", bufs=2, space="PSUM") as psO,
                tc.tile_pool(name="psDen", bufs=2, space="PSUM") as psDen,
            ):
                for J in range(NJ):
                    nlive = 4 * J + 4
                    for h in range(HPC):
                        dqb = at.tile([128, 512], BF, tag="dqb")
                        nc.gpsimd.partition_broadcast(
                            dqb[:], qsrT[0:1, h, J * 512:(J + 1) * 512])
                        oT_ps = psO.tile([128, 512], FP, tag="o")
                        den_ps = psDen.tile([1, 512], FP, tag="den")
                        for ti in range(nlive):
                            sc_ps = psSC.tile([128, 512], FP, tag="sc")
                            nc.tensor.matmul(sc_ps[:], kT[:, ti * 128:(ti + 1) * 128],
                                             qT[:, h, J * 512:(J + 1) * 512])
                            ptf = atf.tile([128, 512], FP, tag="ptf")
                            nc.vector.scalar_tensor_tensor(
                                out=ptf[:], in0=sc_ps[:], scalar=ksr[:, ti:ti + 1],
                                in1=dqb[:], op0=AL.mult, op1=AL.mult)
                            pt = atf.tile([128, 512], BF, tag="pt")
                            nc.scalar.activation(pt[:], ptf[:], AF.Exp)
                            if ti >= 4 * J:
                                nc.gpsimd.affine_select(
                                    out=pt[:], in_=pt[:],
                                    compare_op=AL.is_ge, fill=0.0,
                                    base=J * 512 - ti * 128, channel_multiplier=-1,
                                    pattern=[[1, 512]])
                            nc.tensor.matmul(den_ps[:], ones1[:], pt[:],
                                             start=(ti == 0), stop=(ti == nlive - 1))
                            nc.tensor.matmul(oT_ps[:], vn[:, ti, :], pt[:],
                                             start=(ti == 0), stop=(ti == nlive - 1))
                        denr = at.tile([1, 512], FP, tag="denr")
                        nc.vector.reciprocal_approx_fast(denr[:], den_ps[:])
                        dnb = at.tile([128, 512], FP, tag="dnb")
                        nc.gpsimd.partition_broadcast(dnb[:], denr[:])
                        nc.vector.tensor_mul(OT[:, h, J * 512:(J + 1) * 512],
                                             oT_ps[:], dnb[:])

            # ---------------- Phase C: output projection (partial sums)
            with (
                tc.tile_pool(name="ost", bufs=2) as ost,
                tc.tile_pool(name="psC", bufs=4, space="PSUM") as psC,
            ):
                for st_i in range(ST):
                    for half in range(2):
                        ot_sb = ost.tile([128, D // 2], BF, tag="ot")
                        for dbl in range(4):
                            db = half * 4 + dbl
                            wo_ps = psC.tile([128, 512], FP, tag="wo")
                            for f in range(HPC):
                                nc.tensor.matmul(wo_ps[:], OT[:, f, st_i * 128:(st_i + 1) * 128],
                                                 wo_r[:, f, db * 512:(db + 1) * 512],
                                                 start=(f == 0), stop=(f == HPC - 1))
                            if db % 2 == 0:
                                nc.scalar.copy(ot_sb[:, dbl * 512:(dbl + 1) * 512], wo_ps[:])
                            else:
                                nc.vector.tensor_copy(ot_sb[:, dbl * 512:(dbl + 1) * 512], wo_ps[:])
                        nc.scalar.dma_start(
                            out_e[st_i * 128:(st_i + 1) * 128,
                                  half * (D // 2):(half + 1) * (D // 2)],
                            ot_sb[:])
            wop_cm.__exit__(None, None, None)

    nc.compile()
    return nc


_CACHE = {}


def kernel(x, Wq, Wk, Wv, Wo, cos, sin):
    x2 = np.ascontiguousarray(np.asarray(x, np.float32).reshape(S, D))
    in_maps = []
    for c in range(NCORES):
        in_maps.append({
            "x": x2,
            "wq": np.ascontiguousarray(Wq[:, c * FQ:(c + 1) * FQ], np.float32),
            "wk": np.ascontiguousarray(Wk[:, c * HD:(c + 1) * HD], np.float32),
            "wv": np.ascontiguousarray(Wv[:, c * HD:(c + 1) * HD], np.float32),
            "wo": np.ascontiguousarray(Wo[c * FQ:(c + 1) * FQ, :], np.float32),
            "cos": np.ascontiguousarray(cos, np.float32),
            "sin": np.ascontiguousarray(sin, np.float32),
        })
    if "nc" not in _CACHE:
        _CACHE["nc"] = build_graph()
    try:
        res = run_bass_kernel_spmd(_CACHE["nc"], in_maps, core_ids=list(range(NCORES)))
    except Exception:
        # transient NRT/device hiccups (e.g. EXEC_UNIT_UNRECOVERABLE) usually
        # clear on a fresh attempt
        import time
        time.sleep(20)
        res = run_bass_kernel_spmd(_CACHE["nc"], in_maps, core_ids=list(range(NCORES)))
    out = np.zeros((S, D), np.float64)
    for r in res.results:
        out += np.asarray(r["out"], np.float64)
    return out.astype(np.float32).reshape(B, S, D)

